# revision 9
# baseline (speedup 1.0000x reference)
"""Additive (Bahdanau) attention on 8 Trainium2 NeuronCores.

reference:
    q_proj  = query @ W_query.T                     # (B, H)
    k_proj  = einsum('bsh,oh->bso', keys, W_key)    # (B, S, O)
    scores  = einsum('bso,o->bs', tanh(q_proj[:,None,:] + k_proj), V[0])
    weights = softmax(scores, axis=-1)              # (B, S)
    context = einsum('bs,bsh->bh', weights, keys)   # (B, H)

Sharding: data-parallel over batch, 4 batches per core, no collectives.

Per-core kernel — single pass over keys (transposed layout), big matmuls in
float32r (full PE rate at free dim 512):
  scores:  k_projT chunk [o=128, s=512] = sum_hc W_keyT[hc,oc].T @ keysT[hc,s]
           on PE; ACT applies tanh(psum + q_proj bias) fused; PE contracts
           V.T @ tanh-tile with V replicated over 128 lhsT columns, so the
           scores psum arrives already broadcast across all 128 partitions
           (M=128 costs the same cycles as M=1).
  softmax: scores are bounded (|s| < ~2), so exp needs no max-subtraction.
           ACT computes exp on the broadcast tile with accum_out giving the
           denominator chunk per-partition; DVE sums chunks + reciprocal.
  context: accumulated online on the Vector engine: tensor_mul of the
           resident keysT tile by the exp tile (replicated via a stride-0
           middle axis), free-dim tensor_reduce, accumulate. No second pass
           over keys, no transposes, no extra broadcasts.

Prologue DMAs are chunked (weights by output-column block) and ordered so the
first k_proj matmuls only wait for ~3MB, not the full 10MB of weights.
"""

import numpy as np

import concourse.bass as bass
import concourse.bacc as bacc
import concourse.mybir as mybir
import concourse.tile as tile
from concourse.bass_utils import run_bass_kernel_spmd

N_CORES = 8
B_GLOBAL, S, H = 32, 2048, 1024
B = B_GLOBAL // N_CORES          # local batches per core
P = 128                          # partitions
HC = H // P                      # 8 contraction chunks
OC = H // P                      # 8 output-hidden chunks
SB = 512                         # s-block (matmul free dim)
NSB = S // SB                    # 4 s-blocks

F32 = mybir.dt.float32
F32R = mybir.dt.float32r
AF = mybir.ActivationFunctionType
ALU = mybir.AluOpType
AX = mybir.AxisListType


def _rep_mid(ap, n):
    """Insert a stride-0 middle axis: [P, F] view -> [P, n, F]."""
    return bass.AP(tensor=ap.tensor, offset=ap.offset,
                   ap=[ap.ap[0], [0, n], ap.ap[1]])


def _build(repeat=1):
    nc = bacc.Bacc("TRN2", target_bir_lowering=False, debug=False,
                   num_devices=N_CORES)

    keysT = nc.dram_tensor("keysT", [B, H, S], F32, kind="ExternalInput").ap()
    wkT = nc.dram_tensor("wkT", [H, H], F32, kind="ExternalInput").ap()
    wqT = nc.dram_tensor("wqT", [H, H], F32, kind="ExternalInput").ap()
    qT = nc.dram_tensor("qT", [H, B], F32, kind="ExternalInput").ap()
    vrep = nc.dram_tensor("vrep", [H, P], F32, kind="ExternalInput").ap()

    ctx_out = nc.dram_tensor("ctx_out", [B, H], F32, kind="ExternalOutput").ap()
    w_out = nc.dram_tensor("w_out", [B, S], F32, kind="ExternalOutput").ap()

    with tile.TileContext(nc) as tc:
        with (
            tc.tile_pool(name="singles", bufs=1) as singles,
            tc.tile_pool(name="kt", bufs=3) as kt_pool,
            tc.tile_pool(name="tt", bufs=3) as t_pool,
            tc.tile_pool(name="pexp", bufs=6) as pexp_pool,
            tc.tile_pool(name="wrow", bufs=2) as w_pool,
            tc.tile_pool(name="acc", bufs=2) as acc_pool,
            tc.tile_pool(name="small", bufs=8) as small,
            tc.tile_pool(name="trash", bufs=1) as trash_pool,
            tc.tile_pool(name="psk", bufs=4, space="PSUM") as psk_pool,
            tc.tile_pool(name="pse", bufs=2, space="PSUM") as pse_pool,
            tc.tile_pool(name="psq", bufs=2, space="PSUM") as psq_pool,
        ):
            # ---- prologue: first keys tile + weights, chunked for overlap ----
            qt_sb = singles.tile([P, HC, B], F32R)
            nc.sync.dma_start(out=qt_sb, in_=qT.bitcast(F32R).rearrange(
                "(hc p) b -> p hc b", p=P))
            kt0 = kt_pool.tile([P, HC, SB], F32R, tag="kt")
            nc.sync.dma_start(
                out=kt0, in_=keysT.bitcast(F32R)[0].rearrange(
                    "(hc p) s -> p hc s", p=P)[:, :, 0:SB])

            wk_sb = singles.tile([P, HC, H], F32R)
            wq_sb = singles.tile([P, HC, H], F32R)
            wk_src = wkT.bitcast(F32R).rearrange("(hc p) o -> p hc o", p=P)
            wq_src = wqT.bitcast(F32R).rearrange("(hc p) o -> p hc o", p=P)
            for oc in range(OC):
                osl = slice(oc * P, (oc + 1) * P)
                nc.sync.dma_start(out=wk_sb[:, :, osl], in_=wk_src[:, :, osl])
                nc.sync.dma_start(out=wq_sb[:, :, osl], in_=wq_src[:, :, osl])
            v_sb = singles.tile([P, OC, P], F32R)
            nc.sync.dma_start(out=v_sb, in_=vrep.bitcast(F32R).rearrange(
                "(oc p) m -> p oc m", p=P))

            # ---- q_proj: q_bias[p, oc, b] = (query @ W_query.T)[b, oc*128+p]
            q_bias = singles.tile([P, OC, B], F32)
            for oc in range(OC):
                psq = psq_pool.tile([P, B], F32, tag="psq")
                osl = slice(oc * P, (oc + 1) * P)
                for hc in range(HC):
                    nc.tensor.matmul(psq, wq_sb[:, hc, osl], qt_sb[:, hc, :],
                                     start=(hc == 0), stop=(hc == HC - 1))
                nc.vector.tensor_copy(q_bias[:, oc, :], psq)

            def body(kt_prefetch=None):
                for b in range(B):
                    acc_a = acc_pool.tile([P, HC], F32)
                    acc_b = acc_pool.tile([P, HC], F32)
                    nc.vector.memset(acc_a, 0.0)
                    acc_cur, acc_nxt = acc_a, acc_b
                    pexps = []
                    lchunks = []
                    for sblk in range(NSB):
                        ssl = slice(sblk * SB, (sblk + 1) * SB)
                        if b == 0 and sblk == 0 and kt_prefetch is not None:
                            kt = kt_prefetch
                        else:
                            kt = kt_pool.tile([P, HC, SB], F32R, tag="kt")
                            nc.sync.dma_start(
                                out=kt, in_=keysT.bitcast(F32R)[b].rearrange(
                                    "(hc p) s -> p hc s", p=P)[:, :, ssl])
                        pse = pse_pool.tile([P, SB], F32)
                        for oc in range(OC):
                            osl = slice(oc * P, (oc + 1) * P)
                            psk = psk_pool.tile([P, SB], F32)
                            for hc in range(HC):
                                nc.tensor.matmul(
                                    psk, wk_sb[:, hc, osl], kt[:, hc, :],
                                    start=(hc == 0), stop=(hc == HC - 1))
                            tt = t_pool.tile([P, SB], F32R)
                            nc.scalar.activation(tt, psk, AF.Tanh,
                                                 bias=q_bias[:, oc, b:b + 1])
                            # scores, pre-broadcast over all 128 partitions
                            nc.tensor.matmul(pse, v_sb[:, oc, :], tt,
                                             start=(oc == 0), stop=(oc == OC - 1))
                        # exp chunk (|scores| < ~2: no max-subtract) + denominator
                        pexp = pexp_pool.tile([P, SB], F32, tag="pexp")
                        lc = small.tile([P, 1], F32, tag="lc")
                        nc.scalar.activation(pexp, pse, AF.Exp, accum_out=lc)
                        pexps.append(pexp)
                        lchunks.append(lc)
                        # online context accumulation on DVE
                        trash = trash_pool.tile([P, HC, SB], F32)
                        nc.vector.tensor_mul(trash, kt.bitcast(F32),
                                             _rep_mid(pexp[:, :], HC))
                        part = small.tile([P, HC], F32, tag="part")
                        nc.vector.tensor_reduce(part, trash, AX.X, ALU.add)
                        nc.vector.tensor_add(acc_nxt, acc_cur, part)
                        acc_cur, acc_nxt = acc_nxt, acc_cur

                    # ---- normalization + outputs for batch b ----
                    l01 = small.tile([P, 1], F32, tag="ls")
                    l23 = small.tile([P, 1], F32, tag="ls")
                    lfull = small.tile([P, 1], F32, tag="ls")
                    nc.vector.tensor_add(l01, lchunks[0], lchunks[1])
                    nc.vector.tensor_add(l23, lchunks[2], lchunks[3])
                    nc.vector.tensor_add(lfull, l01, l23)
                    rl_bc = small.tile([P, 1], F32, tag="rlb")
                    nc.vector.reciprocal(rl_bc, lfull)
                    wn = w_pool.tile([1, S], F32)
                    for sblk in range(NSB):
                        ssl = slice(sblk * SB, (sblk + 1) * SB)
                        nc.vector.tensor_scalar_mul(
                            wn[0:1, ssl], pexps[sblk][0:1, :], rl_bc[0:1, 0:1])
                    nc.sync.dma_start(out=w_out[b:b + 1, :], in_=wn)
                    ctx_sb = small.tile([P, HC], F32, tag="ctx")
                    nc.vector.tensor_scalar_mul(ctx_sb, acc_cur, rl_bc)
                    nc.sync.dma_start(
                        out=ctx_out[b:b + 1, :].rearrange(
                            "one (hc p) -> (one p) hc", p=P),
                        in_=ctx_sb)

            body(kt_prefetch=kt0)
            if repeat > 1:
                with tc.For_i(0, repeat - 1, 1):
                    body()

    nc.compile()
    return nc


_compiled_nc = None


def _in_maps(query, keys, W_query, W_key, V):
    query = np.ascontiguousarray(np.asarray(query, dtype=np.float32))
    keys = np.ascontiguousarray(np.asarray(keys, dtype=np.float32))
    wkT = np.ascontiguousarray(np.asarray(W_key, dtype=np.float32).T)
    wqT = np.ascontiguousarray(np.asarray(W_query, dtype=np.float32).T)
    vrep = np.ascontiguousarray(
        np.repeat(np.asarray(V, dtype=np.float32).reshape(H, 1), P, axis=1))
    maps = []
    for c in range(N_CORES):
        kb = keys[c * B:(c + 1) * B]
        maps.append({
            "keysT": np.ascontiguousarray(kb.transpose(0, 2, 1)),
            "wkT": wkT,
            "wqT": wqT,
            "qT": np.ascontiguousarray(query[c * B:(c + 1) * B].T),
            "vrep": vrep,
        })
    return maps


def kernel(query, keys, W_query, W_key, V, _trace=False, _trace_kwargs=None):
    global _compiled_nc
    if _compiled_nc is None:
        _compiled_nc = _build()
    maps = _in_maps(query, keys, W_query, W_key, V)
    out = run_bass_kernel_spmd(_compiled_nc, maps, list(range(N_CORES)),
                               trace=_trace, **(_trace_kwargs or {}))
    res = out.results
    context = np.concatenate([res[c]["ctx_out"] for c in range(N_CORES)], axis=0)
    weights = np.concatenate([res[c]["w_out"] for c in range(N_CORES)], axis=0)
    if _trace:
        return (context, weights), out
    return context, weights


# revision 10
# speedup vs baseline: 424.8385x; 424.8385x over previous
"""Additive (Bahdanau) attention on 8 Trainium2 NeuronCores.

reference:
    q_proj  = query @ W_query.T                     # (B, H)
    k_proj  = einsum('bsh,oh->bso', keys, W_key)    # (B, S, O)
    scores  = einsum('bso,o->bs', tanh(q_proj[:,None,:] + k_proj), V[0])
    weights = softmax(scores, axis=-1)              # (B, S)
    context = einsum('bs,bsh->bh', weights, keys)   # (B, H)

Sharding: data-parallel over batch, 4 batches per core, no collectives.

Per-core kernel — single pass over keys (transposed layout), big matmuls in
float32r (full PE rate at free dim 512):
  scores:  k_projT chunk [o=128, s=512] = sum_hc W_keyT[hc,oc].T @ keysT[hc,s]
           on PE; ACT applies tanh(psum + q_proj bias) fused; PE contracts
           V.T @ tanh-tile with V replicated over 128 lhsT columns, so the
           scores psum arrives already broadcast across all 128 partitions
           (M=128 costs the same cycles as M=1).
  softmax: scores are bounded (|s| < ~2), so exp needs no max-subtraction.
           ACT computes exp on the broadcast tile with accum_out giving the
           denominator chunk per-partition; DVE sums chunks + reciprocal.
  context: accumulated online on the Vector engine: tensor_mul of the
           resident keysT tile by the exp tile (replicated via a stride-0
           middle axis), free-dim tensor_reduce, accumulate. No second pass
           over keys, no transposes, no extra broadcasts.

Prologue DMAs are chunked (weights by output-column block) and ordered so the
first k_proj matmuls only wait for ~3MB, not the full 10MB of weights.
"""

import numpy as np

import concourse.bass as bass
import concourse.bacc as bacc
import concourse.mybir as mybir
import concourse.tile as tile
from concourse.bass_utils import run_bass_kernel_spmd

N_CORES = 8
B_GLOBAL, S, H = 32, 2048, 1024
B = B_GLOBAL // N_CORES          # local batches per core
P = 128                          # partitions
HC = H // P                      # 8 contraction chunks
OC = H // P                      # 8 output-hidden chunks
SB = 512                         # s-block (matmul free dim)
NSB = S // SB                    # 4 s-blocks

F32 = mybir.dt.float32
F32R = mybir.dt.float32r
AF = mybir.ActivationFunctionType
ALU = mybir.AluOpType
AX = mybir.AxisListType


def _rep_mid(ap, n):
    """Insert a stride-0 middle axis: [P, F] view -> [P, n, F]."""
    return bass.AP(tensor=ap.tensor, offset=ap.offset,
                   ap=[ap.ap[0], [0, n], ap.ap[1]])


def _build(repeat=1):
    nc = bacc.Bacc("TRN2", target_bir_lowering=False, debug=False,
                   num_devices=N_CORES)

    keysT = nc.dram_tensor("keysT", [B, H, S], F32, kind="ExternalInput").ap()
    wkT = nc.dram_tensor("wkT", [H, H], F32, kind="ExternalInput").ap()
    wqT = nc.dram_tensor("wqT", [H, H], F32, kind="ExternalInput").ap()
    qT = nc.dram_tensor("qT", [H, B], F32, kind="ExternalInput").ap()
    vrep = nc.dram_tensor("vrep", [H, P], F32, kind="ExternalInput").ap()

    ctx_out = nc.dram_tensor("ctx_out", [B, H], F32, kind="ExternalOutput").ap()
    w_out = nc.dram_tensor("w_out", [B, S], F32, kind="ExternalOutput").ap()

    with tile.TileContext(nc) as tc:
        with (
            tc.tile_pool(name="singles", bufs=1) as singles,
            tc.tile_pool(name="kt", bufs=3) as kt_pool,
            tc.tile_pool(name="tt", bufs=3) as t_pool,
            tc.tile_pool(name="pexp", bufs=6) as pexp_pool,
            tc.tile_pool(name="wrow", bufs=2) as w_pool,
            tc.tile_pool(name="acc", bufs=2) as acc_pool,
            tc.tile_pool(name="small", bufs=8) as small,
            tc.tile_pool(name="trash", bufs=1) as trash_pool,
            tc.tile_pool(name="psk", bufs=4, space="PSUM") as psk_pool,
            tc.tile_pool(name="pse", bufs=2, space="PSUM") as pse_pool,
            tc.tile_pool(name="psq", bufs=2, space="PSUM") as psq_pool,
        ):
            # ---- prologue: first keys tile + weights, chunked for overlap ----
            qt_sb = singles.tile([P, HC, B], F32R)
            nc.sync.dma_start(out=qt_sb, in_=qT.bitcast(F32R).rearrange(
                "(hc p) b -> p hc b", p=P))
            kt0 = kt_pool.tile([P, HC, SB], F32R, tag="kt")
            kt0_src = keysT.bitcast(F32R)[0].rearrange(
                "(hc p) s -> p hc s", p=P)[:, :, 0:SB]
            for hq in range(4):
                nc.sync.dma_start(out=kt0[:, 2 * hq:2 * hq + 2, :],
                                  in_=kt0_src[:, 2 * hq:2 * hq + 2, :])

            wk_sb = singles.tile([P, HC, H], F32R)
            wq_sb = singles.tile([P, HC, H], F32R)
            wk_src = wkT.bitcast(F32R).rearrange("(hc p) o -> p hc o", p=P)
            wq_src = wqT.bitcast(F32R).rearrange("(hc p) o -> p hc o", p=P)
            for oc in range(OC):
                osl = slice(oc * P, (oc + 1) * P)
                nc.sync.dma_start(out=wk_sb[:, :, osl], in_=wk_src[:, :, osl])
                nc.sync.dma_start(out=wq_sb[:, :, osl], in_=wq_src[:, :, osl])
            v_sb = singles.tile([P, OC, P], F32R)
            nc.sync.dma_start(out=v_sb, in_=vrep.bitcast(F32R).rearrange(
                "(oc p) m -> p oc m", p=P))

            # ---- q_proj: q_bias[p, oc, b] = (query @ W_query.T)[b, oc*128+p]
            q_bias = singles.tile([P, OC, B], F32)
            for oc in range(OC):
                psq = psq_pool.tile([P, B], F32, tag="psq")
                osl = slice(oc * P, (oc + 1) * P)
                for hc in range(HC):
                    nc.tensor.matmul(psq, wq_sb[:, hc, osl], qt_sb[:, hc, :],
                                     start=(hc == 0), stop=(hc == HC - 1))
                nc.vector.tensor_copy(q_bias[:, oc, :], psq)

            def body(kt_prefetch=None):
                for b in range(B):
                    acc_a = acc_pool.tile([P, HC], F32)
                    acc_b = acc_pool.tile([P, HC], F32)
                    nc.vector.memset(acc_a, 0.0)
                    acc_cur, acc_nxt = acc_a, acc_b
                    pexps = []
                    lchunks = []
                    for sblk in range(NSB):
                        ssl = slice(sblk * SB, (sblk + 1) * SB)
                        if b == 0 and sblk == 0 and kt_prefetch is not None:
                            kt = kt_prefetch
                        else:
                            kt = kt_pool.tile([P, HC, SB], F32R, tag="kt")
                            kt_src = keysT.bitcast(F32R)[b].rearrange(
                                "(hc p) s -> p hc s", p=P)[:, :, ssl]
                            for hh in range(2):
                                nc.sync.dma_start(
                                    out=kt[:, 4 * hh:4 * hh + 4, :],
                                    in_=kt_src[:, 4 * hh:4 * hh + 4, :])
                        pse = pse_pool.tile([P, SB], F32)
                        for oc in range(OC):
                            osl = slice(oc * P, (oc + 1) * P)
                            psk = psk_pool.tile([P, SB], F32)
                            for hc in range(HC):
                                nc.tensor.matmul(
                                    psk, wk_sb[:, hc, osl], kt[:, hc, :],
                                    start=(hc == 0), stop=(hc == HC - 1))
                            tt = t_pool.tile([P, SB], F32R)
                            nc.scalar.activation(tt, psk, AF.Tanh,
                                                 bias=q_bias[:, oc, b:b + 1])
                            # scores, pre-broadcast over all 128 partitions
                            nc.tensor.matmul(pse, v_sb[:, oc, :], tt,
                                             start=(oc == 0), stop=(oc == OC - 1))
                        # exp chunk (|scores| < ~2: no max-subtract) + denominator
                        pexp = pexp_pool.tile([P, SB], F32, tag="pexp")
                        lc = small.tile([P, 1], F32, tag="lc")
                        nc.scalar.activation(pexp, pse, AF.Exp, accum_out=lc)
                        pexps.append(pexp)
                        lchunks.append(lc)
                        # online context accumulation on DVE
                        trash = trash_pool.tile([P, HC, SB], F32)
                        nc.vector.tensor_mul(trash, kt.bitcast(F32),
                                             _rep_mid(pexp[:, :], HC))
                        part = small.tile([P, HC], F32, tag="part")
                        nc.vector.tensor_reduce(part, trash, AX.X, ALU.add)
                        nc.vector.tensor_add(acc_nxt, acc_cur, part)
                        acc_cur, acc_nxt = acc_nxt, acc_cur

                    # ---- normalization + outputs for batch b ----
                    l01 = small.tile([P, 1], F32, tag="ls")
                    l23 = small.tile([P, 1], F32, tag="ls")
                    lfull = small.tile([P, 1], F32, tag="ls")
                    nc.vector.tensor_add(l01, lchunks[0], lchunks[1])
                    nc.vector.tensor_add(l23, lchunks[2], lchunks[3])
                    nc.vector.tensor_add(lfull, l01, l23)
                    rl_bc = small.tile([P, 1], F32, tag="rlb")
                    nc.vector.reciprocal(rl_bc, lfull)
                    wn = w_pool.tile([1, S], F32)
                    for sblk in range(NSB):
                        ssl = slice(sblk * SB, (sblk + 1) * SB)
                        nc.vector.tensor_scalar_mul(
                            wn[0:1, ssl], pexps[sblk][0:1, :], rl_bc[0:1, 0:1])
                    nc.sync.dma_start(out=w_out[b:b + 1, :], in_=wn)
                    ctx_sb = small.tile([P, HC], F32, tag="ctx")
                    nc.vector.tensor_scalar_mul(ctx_sb, acc_cur, rl_bc)
                    nc.sync.dma_start(
                        out=ctx_out[b:b + 1, :].rearrange(
                            "one (hc p) -> (one p) hc", p=P),
                        in_=ctx_sb)

            body(kt_prefetch=kt0)
            if repeat > 1:
                with tc.For_i(0, repeat - 1, 1):
                    body()

    nc.compile()
    return nc


_compiled_nc = None


def _in_maps(query, keys, W_query, W_key, V):
    query = np.ascontiguousarray(np.asarray(query, dtype=np.float32))
    keys = np.ascontiguousarray(np.asarray(keys, dtype=np.float32))
    wkT = np.ascontiguousarray(np.asarray(W_key, dtype=np.float32).T)
    wqT = np.ascontiguousarray(np.asarray(W_query, dtype=np.float32).T)
    vrep = np.ascontiguousarray(
        np.repeat(np.asarray(V, dtype=np.float32).reshape(H, 1), P, axis=1))
    maps = []
    for c in range(N_CORES):
        kb = keys[c * B:(c + 1) * B]
        maps.append({
            "keysT": np.ascontiguousarray(kb.transpose(0, 2, 1)),
            "wkT": wkT,
            "wqT": wqT,
            "qT": np.ascontiguousarray(query[c * B:(c + 1) * B].T),
            "vrep": vrep,
        })
    return maps


def kernel(query, keys, W_query, W_key, V, _trace=False, _trace_kwargs=None):
    global _compiled_nc
    if _compiled_nc is None:
        _compiled_nc = _build()
    maps = _in_maps(query, keys, W_query, W_key, V)
    out = run_bass_kernel_spmd(_compiled_nc, maps, list(range(N_CORES)),
                               trace=_trace, **(_trace_kwargs or {}))
    res = out.results
    context = np.concatenate([res[c]["ctx_out"] for c in range(N_CORES)], axis=0)
    weights = np.concatenate([res[c]["w_out"] for c in range(N_CORES)], axis=0)
    if _trace:
        return (context, weights), out
    return context, weights


# revision 13
# speedup vs baseline: 443.8279x; 1.0447x over previous
"""Additive (Bahdanau) attention on 8 Trainium2 NeuronCores.

reference:
    q_proj  = query @ W_query.T                     # (B, H)
    k_proj  = einsum('bsh,oh->bso', keys, W_key)    # (B, S, O)
    scores  = einsum('bso,o->bs', tanh(q_proj[:,None,:] + k_proj), V[0])
    weights = softmax(scores, axis=-1)              # (B, S)
    context = einsum('bs,bsh->bh', weights, keys)   # (B, H)

Sharding: data-parallel over batch, 4 batches per core, no collectives.

Per-core kernel — single pass over keys (transposed layout), big matmuls in
float32r (full PE rate at free dim 512):
  scores:  k_projT chunk [o=128, s=512] = sum_hc W_keyT[hc,oc].T @ keysT[hc,s]
           on PE; ACT applies tanh(psum + q_proj bias) fused; PE contracts
           V.T @ tanh-tile with V replicated over 128 lhsT columns, so the
           scores psum arrives already broadcast across all 128 partitions
           (M=128 costs the same cycles as M=1).
  softmax: scores are bounded (|s| < ~2), so exp needs no max-subtraction.
           ACT computes exp on the broadcast tile with accum_out giving the
           denominator chunk per-partition; DVE sums chunks + reciprocal.
  context: accumulated online on the Vector engine: tensor_mul of the
           resident keysT tile by the exp tile (replicated via a stride-0
           middle axis), free-dim tensor_reduce, accumulate. No second pass
           over keys, no transposes, no extra broadcasts.

Prologue DMAs are chunked (weights by output-column block) and ordered so the
first k_proj matmuls only wait for ~3MB, not the full 10MB of weights.
"""

import numpy as np

import concourse.bass as bass
import concourse.bacc as bacc
import concourse.mybir as mybir
import concourse.tile as tile
from concourse.bass_utils import run_bass_kernel_spmd

N_CORES = 8
B_GLOBAL, S, H = 32, 2048, 1024
B = B_GLOBAL // N_CORES          # local batches per core
P = 128                          # partitions
HC = H // P                      # 8 contraction chunks
OC = H // P                      # 8 output-hidden chunks
SB = 512                         # s-block (matmul free dim)
NSB = S // SB                    # 4 s-blocks

F32 = mybir.dt.float32
F32R = mybir.dt.float32r
F16 = mybir.dt.float16
AF = mybir.ActivationFunctionType
ALU = mybir.AluOpType
AX = mybir.AxisListType


def _rep_mid(ap, n):
    """Insert a stride-0 middle axis: [P, F] view -> [P, n, F]."""
    return bass.AP(tensor=ap.tensor, offset=ap.offset,
                   ap=[ap.ap[0], [0, n], ap.ap[1]])


def _build(repeat=1):
    nc = bacc.Bacc("TRN2", target_bir_lowering=False, debug=False,
                   num_devices=N_CORES)

    keysT = nc.dram_tensor("keysT", [B, H, S], F16, kind="ExternalInput").ap()
    wkT = nc.dram_tensor("wkT", [H, H], F16, kind="ExternalInput").ap()
    wqT = nc.dram_tensor("wqT", [H, H], F16, kind="ExternalInput").ap()
    qT = nc.dram_tensor("qT", [H, B], F16, kind="ExternalInput").ap()
    vrep = nc.dram_tensor("vrep", [H, P], F16, kind="ExternalInput").ap()

    ctx_out = nc.dram_tensor("ctx_out", [B, H], F32, kind="ExternalOutput").ap()
    w_out = nc.dram_tensor("w_out", [B, S], F32, kind="ExternalOutput").ap()

    with tile.TileContext(nc) as tc:
        with (
            tc.tile_pool(name="singles", bufs=1) as singles,
            tc.tile_pool(name="kt", bufs=3) as kt_pool,
            tc.tile_pool(name="tt", bufs=3) as t_pool,
            tc.tile_pool(name="pexp", bufs=6) as pexp_pool,
            tc.tile_pool(name="wrow", bufs=2) as w_pool,
            tc.tile_pool(name="acc", bufs=2) as acc_pool,
            tc.tile_pool(name="small", bufs=8) as small,
            tc.tile_pool(name="trash", bufs=1) as trash_pool,
            tc.tile_pool(name="psk", bufs=4, space="PSUM") as psk_pool,
            tc.tile_pool(name="pse", bufs=2, space="PSUM") as pse_pool,
            tc.tile_pool(name="psq", bufs=2, space="PSUM") as psq_pool,
        ):
            # ---- prologue: first keys tile + weights, chunked for overlap ----
            qt_sb = singles.tile([P, HC, B], F16)
            nc.sync.dma_start(out=qt_sb, in_=qT.rearrange(
                "(hc p) b -> p hc b", p=P))
            kt0 = kt_pool.tile([P, HC, SB], F16, tag="kt")
            kt0_src = keysT[0].rearrange(
                "(hc p) s -> p hc s", p=P)[:, :, 0:SB]
            for hq in range(4):
                nc.sync.dma_start(out=kt0[:, 2 * hq:2 * hq + 2, :],
                                  in_=kt0_src[:, 2 * hq:2 * hq + 2, :])

            wk_sb = singles.tile([P, HC, H], F16)
            wq_sb = singles.tile([P, HC, H], F16)
            wk_src = wkT.rearrange("(hc p) o -> p hc o", p=P)
            wq_src = wqT.rearrange("(hc p) o -> p hc o", p=P)
            for oc in range(OC):
                osl = slice(oc * P, (oc + 1) * P)
                nc.sync.dma_start(out=wk_sb[:, :, osl], in_=wk_src[:, :, osl])
                nc.sync.dma_start(out=wq_sb[:, :, osl], in_=wq_src[:, :, osl])
            v_sb = singles.tile([P, OC, P], F16)
            nc.sync.dma_start(out=v_sb, in_=vrep.rearrange(
                "(oc p) m -> p oc m", p=P))

            # ---- q_proj: q_bias[p, oc, b] = (query @ W_query.T)[b, oc*128+p]
            q_bias = singles.tile([P, OC, B], F32)
            for oc in range(OC):
                psq = psq_pool.tile([P, B], F32, tag="psq")
                osl = slice(oc * P, (oc + 1) * P)
                for hc in range(HC):
                    nc.tensor.matmul(psq, wq_sb[:, hc, osl], qt_sb[:, hc, :],
                                     start=(hc == 0), stop=(hc == HC - 1))
                nc.vector.tensor_copy(q_bias[:, oc, :], psq)

            def body(kt_prefetch=None):
                for b in range(B):
                    acc_a = acc_pool.tile([P, HC], F32)
                    acc_b = acc_pool.tile([P, HC], F32)
                    nc.vector.memset(acc_a, 0.0)
                    acc_cur, acc_nxt = acc_a, acc_b
                    pexps = []
                    lchunks = []
                    for sblk in range(NSB):
                        ssl = slice(sblk * SB, (sblk + 1) * SB)
                        if b == 0 and sblk == 0 and kt_prefetch is not None:
                            kt = kt_prefetch
                        else:
                            kt = kt_pool.tile([P, HC, SB], F16, tag="kt")
                            kt_src = keysT[b].rearrange(
                                "(hc p) s -> p hc s", p=P)[:, :, ssl]
                            for hh in range(2):
                                nc.sync.dma_start(
                                    out=kt[:, 4 * hh:4 * hh + 4, :],
                                    in_=kt_src[:, 4 * hh:4 * hh + 4, :])
                        pse = pse_pool.tile([P, SB], F32)
                        for oc in range(OC):
                            osl = slice(oc * P, (oc + 1) * P)
                            psk = psk_pool.tile([P, SB], F32)
                            for hc in range(HC):
                                nc.tensor.matmul(
                                    psk, wk_sb[:, hc, osl], kt[:, hc, :],
                                    start=(hc == 0), stop=(hc == HC - 1))
                            tt = t_pool.tile([P, SB], F16)
                            nc.scalar.activation(tt, psk, AF.Tanh,
                                                 bias=q_bias[:, oc, b:b + 1])
                            # scores, pre-broadcast over all 128 partitions
                            nc.tensor.matmul(pse, v_sb[:, oc, :], tt,
                                             start=(oc == 0), stop=(oc == OC - 1))
                        # exp chunk (|scores| < ~2: no max-subtract) + denominator
                        pexp = pexp_pool.tile([P, SB], F32, tag="pexp")
                        lc = small.tile([P, 1], F32, tag="lc")
                        nc.scalar.activation(pexp, pse, AF.Exp, accum_out=lc)
                        pexps.append(pexp)
                        lchunks.append(lc)
                        # online context accumulation on DVE
                        trash = trash_pool.tile([P, HC, SB], F32)
                        nc.vector.tensor_mul(trash, kt,
                                             _rep_mid(pexp[:, :], HC))
                        part = small.tile([P, HC], F32, tag="part")
                        nc.vector.tensor_reduce(part, trash, AX.X, ALU.add)
                        nc.vector.tensor_add(acc_nxt, acc_cur, part)
                        acc_cur, acc_nxt = acc_nxt, acc_cur

                    # ---- normalization + outputs for batch b ----
                    l01 = small.tile([P, 1], F32, tag="ls")
                    l23 = small.tile([P, 1], F32, tag="ls")
                    lfull = small.tile([P, 1], F32, tag="ls")
                    nc.vector.tensor_add(l01, lchunks[0], lchunks[1])
                    nc.vector.tensor_add(l23, lchunks[2], lchunks[3])
                    nc.vector.tensor_add(lfull, l01, l23)
                    rl_bc = small.tile([P, 1], F32, tag="rlb")
                    nc.vector.reciprocal(rl_bc, lfull)
                    wn = w_pool.tile([1, S], F32)
                    for sblk in range(NSB):
                        ssl = slice(sblk * SB, (sblk + 1) * SB)
                        nc.vector.tensor_scalar_mul(
                            wn[0:1, ssl], pexps[sblk][0:1, :], rl_bc[0:1, 0:1])
                    nc.sync.dma_start(out=w_out[b:b + 1, :], in_=wn)
                    ctx_sb = small.tile([P, HC], F32, tag="ctx")
                    nc.vector.tensor_scalar_mul(ctx_sb, acc_cur, rl_bc)
                    nc.sync.dma_start(
                        out=ctx_out[b:b + 1, :].rearrange(
                            "one (hc p) -> (one p) hc", p=P),
                        in_=ctx_sb)

            body(kt_prefetch=kt0)
            if repeat > 1:
                with tc.For_i(0, repeat - 1, 1):
                    body()

    nc.compile()
    return nc


_compiled_nc = None


def _in_maps(query, keys, W_query, W_key, V):
    query = np.ascontiguousarray(np.asarray(query, dtype=np.float32))
    keys = np.ascontiguousarray(np.asarray(keys, dtype=np.float32))
    wkT = np.ascontiguousarray(np.asarray(W_key, dtype=np.float32).T.astype(np.float16))
    wqT = np.ascontiguousarray(np.asarray(W_query, dtype=np.float32).T.astype(np.float16))
    vrep = np.ascontiguousarray(
        np.repeat(np.asarray(V, dtype=np.float32).reshape(H, 1), P,
                  axis=1).astype(np.float16))
    maps = []
    for c in range(N_CORES):
        kb = keys[c * B:(c + 1) * B]
        maps.append({
            "keysT": np.ascontiguousarray(kb.transpose(0, 2, 1).astype(np.float16)),
            "wkT": wkT,
            "wqT": wqT,
            "qT": np.ascontiguousarray(query[c * B:(c + 1) * B].T.astype(np.float16)),
            "vrep": vrep,
        })
    return maps


def kernel(query, keys, W_query, W_key, V, _trace=False, _trace_kwargs=None):
    global _compiled_nc
    if _compiled_nc is None:
        _compiled_nc = _build()
    maps = _in_maps(query, keys, W_query, W_key, V)
    out = run_bass_kernel_spmd(_compiled_nc, maps, list(range(N_CORES)),
                               trace=_trace, **(_trace_kwargs or {}))
    res = out.results
    context = np.concatenate([res[c]["ctx_out"] for c in range(N_CORES)], axis=0)
    weights = np.concatenate([res[c]["w_out"] for c in range(N_CORES)], axis=0)
    if _trace:
        return (context, weights), out
    return context, weights
